# revision 2
# baseline (speedup 1.0000x reference)
"""Trainium2 Bass kernel for nn_CSFI_26182120636676.

FrequencyChannelAttention + FrequencySpatialAttention over x [4, 128, 448, 448] f32.

Two SPMD launches on 8 NeuronCores:
  Pass A (channel-sharded, 16 ch/core): 64x64 block-sum pooling -> [7,7] sums
    per (b, c). DVE segmented reduce over w + tiny PE indicator matmul over h.
    Host finishes the tiny SE MLP (sigmoid gate y[b, c]).
  Pass B (batch x h-half sharded): per 8-row strip, 8 accumulating fp32r
    matmuls fold the channel contraction (with v = y*wc) AND the first 8-pt
    DCT stage; PE transposes move data to w-partitions; a block-diagonal
    matmul applies the second DCT stage; mask*wp multiply; single-column
    transposes collect gate rows at partition 0; ACT sigmoid; rank-1 fp32r
    matmuls broadcast y[c]*gate to 128 partitions; one DVE multiply forms
    out = x * y * gate.
"""
import math
import numpy as np

import concourse.bacc as bacc
import concourse.tile as tile
from concourse import mybir
from concourse.bass_utils import run_bass_kernel_spmd

F32 = mybir.dt.float32
F32R = mybir.dt.float32r

C = 128
H = W = 448
DCT_H = DCT_W = 7
NUM_FREQ = 16
BH = H // DCT_H  # 64

_TOP_X = [0, 0, 6, 0, 0, 1, 1, 4, 5, 1, 3, 0, 0, 0, 3, 2]
_TOP_Y = [0, 1, 0, 5, 2, 0, 2, 0, 0, 6, 0, 4, 6, 3, 5, 2]


def _basis(pos, freq, P):
    v = math.cos(math.pi * freq * (pos + 0.5) / P) / math.sqrt(P)
    return v if freq == 0 else v * math.sqrt(2)


def fca_filter():
    filt = np.zeros((C, DCT_H, DCT_W), np.float32)
    cp = C // NUM_FREQ
    for i, (ux, vy) in enumerate(zip(_TOP_X, _TOP_Y)):
        for xx in range(DCT_H):
            for yy in range(DCT_W):
                filt[i * cp:(i + 1) * cp, xx, yy] = _basis(xx, ux, DCT_H) * _basis(yy, vy, DCT_W)
    return filt


def dct1d(N):
    k = np.arange(N)[:, None].astype(np.float64)
    n = np.arange(N)[None, :].astype(np.float64)
    m = np.cos(np.pi / N * (n + 0.5) * k)
    m[0] /= math.sqrt(N)
    m[1:] /= math.sqrt(N / 2)
    return m.astype(np.float32)


def zigzag_mask(h, w, nf):
    coords = []
    for s in range(h + w - 1):
        if s % 2 == 0:
            r, c = min(s, h - 1), s - min(s, h - 1)
            while r >= 0 and c < w:
                coords.append((r, c)); r -= 1; c += 1
        else:
            c, r = min(s, w - 1), s - min(s, w - 1)
            while c >= 0 and r < h:
                coords.append((r, c)); r += 1; c -= 1
    mask = np.zeros(h * w, np.float32)
    for (r, c) in coords[:nf]:
        mask[r * w + c] = 1.0
    return mask


D8 = dct1d(8)
MASK8 = zigzag_mask(8, 8, NUM_FREQ).reshape(8, 8)
FILT = fca_filter()


def host_mlp(pooled_sums, w1, w2, wc):
    """pooled_sums [4, 128, 7, 7] block SUMS -> gate y [4, 128], v = y*wc [4, 128]."""
    pooled = pooled_sums.astype(np.float32) / (BH * BH)
    y_pre = np.einsum('bchw,chw->bc', pooled, FILT)
    h1 = np.maximum(y_pre @ w1.T, 0.0)
    y = 1.0 / (1.0 + np.exp(-(h1 @ w2.T)))
    v = y * wc[0][None, :]
    return y.astype(np.float32), v.astype(np.float32)


def passA_consts():
    ind = np.zeros((128, 32), np.float32)
    for t in range(4):
        for p in range(128):
            h = 128 * t + p
            if h < H:
                ind[p, t * 8 + (h // BH)] = 1.0
    return ind


def passB_consts(wpar):
    b1 = np.zeros((112, 112), np.float32)
    for wpl in range(14):
        b1[wpl * 8:(wpl + 1) * 8, wpl * 8:(wpl + 1) * 8] = D8.T
    maskw = np.zeros((112, 32), np.float32)
    for wpl in range(14):
        for v in range(8):
            for ch in range(4):
                for u in range(8):
                    maskw[wpl * 8 + v, ch * 8 + u] = wpar * MASK8[u, v]
    id8 = np.eye(8, dtype=np.float32)
    id112 = np.eye(112, dtype=np.float32)
    return b1, maskw, id8, id112


def passB_vd8(v_b):
    out = np.zeros((128, 64), np.float32)
    for ph in range(8):
        out[:, ph * 8:(ph + 1) * 8] = v_b[:, None] * D8[:, ph][None, :]
    return out


def build_passA(n_loop=1):
    nc = bacc.Bacc("TRN2", target_bir_lowering=False, debug=False)
    xa = nc.dram_tensor("xa", [4, 16, H, W], F32, kind="ExternalInput").ap()
    ind_d = nc.dram_tensor("ind", [128, 32], F32, kind="ExternalInput").ap()
    pooled_d = nc.dram_tensor("pooled", [7, 448], F32, kind="ExternalOutput").ap()

    with tile.TileContext(nc) as tc:
        with tc.tile_pool(name="consts", bufs=1) as cpool, \
             tc.tile_pool(name="xin", bufs=4) as xpool, \
             tc.tile_pool(name="rs", bufs=2) as rspool, \
             tc.tile_pool(name="psb", bufs=1) as psbpool, \
             tc.tile_pool(name="pp", bufs=2, space="PSUM") as ppool:
            ind = cpool.tile([128, 32], F32)
            nc.sync.dma_start(ind[:], ind_d[:])

            import contextlib
            loop_cm = tc.For_i(0, n_loop, 1) if n_loop > 1 else contextlib.nullcontext()
            with loop_cm:
                psb = psbpool.tile([7, 448], F32, tag="psb")
                for pair in range(64):
                    b, cl = pair // 16, pair % 16
                    pooled_ps = ppool.tile([8, 7], F32, tag="pp")
                    for t in range(4):
                        rows = 128 if t < 3 else 64
                        xt = xpool.tile([128, 448], F32, tag="x")
                        nc.sync.dma_start(
                            xt[0:rows, :], xa[b, cl, 128 * t:128 * t + rows, :])
                        rs = rspool.tile([128, 7], F32, tag="rs")
                        nc.vector.reduce_sum(
                            rs[0:rows, :],
                            xt[0:rows, :].rearrange("p (j k) -> p j k", j=7),
                            axis=mybir.AxisListType.X)
                        nc.tensor.matmul(
                            pooled_ps[:], ind[:, 8 * t:8 * t + 8], rs[:],
                            start=(t == 0), stop=(t == 3))
                    nc.scalar.copy(psb[0:7, 7 * pair:7 * pair + 7], pooled_ps[0:7, :])
                nc.sync.dma_start(pooled_d[:], psb[:])
    nc.compile()
    return nc


def build_passB(n_strips=28, n_loop=1):
    HS = n_strips * 8
    nc = bacc.Bacc("TRN2", target_bir_lowering=False, debug=False)
    xb = nc.dram_tensor("xb", [128, HS, W], F32R, kind="ExternalInput").ap()
    vd8_d = nc.dram_tensor("vd8", [128, 64], F32R, kind="ExternalInput").ap()
    yrow_d = nc.dram_tensor("yrow", [1, 128], F32R, kind="ExternalInput").ap()
    b1_d = nc.dram_tensor("b1", [112, 112], F32, kind="ExternalInput").ap()
    maskw_d = nc.dram_tensor("maskw", [112, 32], F32, kind="ExternalInput").ap()
    id8_d = nc.dram_tensor("id8", [8, 8], F32, kind="ExternalInput").ap()
    id112_d = nc.dram_tensor("id112", [112, 112], F32, kind="ExternalInput").ap()
    out_d = nc.dram_tensor("out", [128, HS, W], F32, kind="ExternalOutput").ap()

    with tile.TileContext(nc) as tc:
        with tc.tile_pool(name="consts", bufs=1) as cpool, \
             tc.tile_pool(name="xin", bufs=3) as xpool, \
             tc.tile_pool(name="outp", bufs=3) as opool, \
             tc.tile_pool(name="small", bufs=2) as spool, \
             tc.tile_pool(name="gat", bufs=2) as gpool, \
             tc.tile_pool(name="ps_t1", bufs=2, space="PSUM") as ps_t1, \
             tc.tile_pool(name="ps_t1t", bufs=1, space="PSUM") as ps_t1t, \
             tc.tile_pool(name="ps_coef", bufs=1, space="PSUM") as ps_coef, \
             tc.tile_pool(name="ps_grow", bufs=2, space="PSUM") as ps_grow, \
             tc.tile_pool(name="ps_gb", bufs=2, space="PSUM") as ps_gb:
            vd8 = cpool.tile([128, 64], F32R)
            nc.sync.dma_start(vd8[:], vd8_d[:])
            yrow = cpool.tile([1, 128], F32R)
            nc.sync.dma_start(yrow[:], yrow_d[:])
            b1 = cpool.tile([112, 112], F32)
            nc.sync.dma_start(b1[:], b1_d[:])
            maskw = cpool.tile([112, 32], F32)
            nc.sync.dma_start(maskw[:], maskw_d[:])
            id8 = cpool.tile([8, 8], F32)
            nc.sync.dma_start(id8[:], id8_d[:])
            id112 = cpool.tile([112, 112], F32)
            nc.sync.dma_start(id112[:], id112_d[:])

            import contextlib
            loop_cm = tc.For_i(0, n_loop, 1) if n_loop > 1 else contextlib.nullcontext()
            with loop_cm:
                for s in range(n_strips):
                    h0 = 8 * s
                    xt = xpool.tile([128, 8, 448], F32R, tag="x")
                    nc.sync.dma_start(xt[:], xb[:, h0:h0 + 8, :])
                    # T1[u, w] = sum_ph sum_c v[c] D8[u, ph] x[c, ph, w]
                    t1 = ps_t1.tile([8, 448], F32, tag="t1")
                    for ph in range(8):
                        nc.tensor.matmul(
                            t1[:],
                            vd8[:, 8 * ph:8 * ph + 8],
                            xt[:, ph, :],
                            start=(ph == 0), stop=(ph == 7))
                    t1_sb = spool.tile([8, 448], F32, tag="t1sb")
                    nc.vector.tensor_copy(t1_sb[:], t1[:])
                    # transpose chunks -> T1T [112 (wpl,pw), 32 (chunk,u)]
                    t1t = ps_t1t.tile([112, 32], F32, tag="t1t")
                    for ch in range(4):
                        nc.tensor.transpose(
                            t1t[:, 8 * ch:8 * ch + 8],
                            t1_sb[:, 112 * ch:112 * ch + 112], id8[:])
                    t1t_sb = spool.tile([112, 32], F32, tag="t1tsb")
                    nc.vector.tensor_copy(t1t_sb[:], t1t[:])
                    # 2nd DCT stage: COEF [112 (wpl,v), 32 (chunk,u)]
                    coef = ps_coef.tile([112, 32], F32, tag="coef")
                    nc.tensor.matmul(coef[:], b1[:], t1t_sb[:], start=True, stop=True)
                    mc = spool.tile([112, 32], F32, tag="mc")
                    nc.vector.tensor_mul(mc[:], coef[:], maskw[:])
                    # gate rows at partition 0: [1, 3584]
                    gatesb = gpool.tile([1, 8 * 448], F32R, tag="gate")
                    for u in range(8):
                        grow = ps_grow.tile([1, 448], F32, tag="grow")
                        for ch in range(4):
                            nc.tensor.transpose(
                                grow[:, 112 * ch:112 * ch + 112],
                                mc[:, 8 * ch + u:8 * ch + u + 1], id112[:])
                        nc.scalar.activation(
                            gatesb[:, 448 * u:448 * (u + 1)], grow[:],
                            mybir.ActivationFunctionType.Sigmoid)
                    # broadcast y[c]*gate to 128 partitions, multiply with x
                    ot = opool.tile([128, 8, 448], F32, tag="o")
                    for u in range(8):
                        gb = ps_gb.tile([128, 448], F32, tag="gb")
                        nc.tensor.matmul(
                            gb[:], yrow[:],
                            gatesb[:, 448 * u:448 * (u + 1)],
                            start=True, stop=True)
                        nc.vector.tensor_mul(
                            ot[:, u, :], xt[:, u, :].bitcast(F32), gb[:])
                    nc.sync.dma_start(out_d[:, h0:h0 + 8, :], ot[:])
    nc.compile()
    return nc


_CACHE = {}


def _get(name, builder):
    if name not in _CACHE:
        _CACHE[name] = builder()
    return _CACHE[name]


def kernel(x, w1, w2, wc, wp):
    x = np.asarray(x, dtype=np.float32)
    w1 = np.asarray(w1, dtype=np.float32)
    w2 = np.asarray(w2, dtype=np.float32)
    wc = np.asarray(wc, dtype=np.float32)
    wp = np.asarray(wp, dtype=np.float32)

    cores = list(range(8))

    # ---- pass A: pooled block sums ----
    nc_a = _get("A", build_passA)
    ind = passA_consts()
    in_maps_a = [
        {"xa": np.ascontiguousarray(x[:, 16 * k:16 * (k + 1)]), "ind": ind}
        for k in cores
    ]
    res_a = run_bass_kernel_spmd(nc_a, in_maps_a, cores)
    pooled = np.zeros((4, 128, 7, 7), np.float32)
    for k in cores:
        pr = res_a.results[k]["pooled"].reshape(7, 64, 7).transpose(1, 0, 2)  # [pair, i, j]
        pooled[:, 16 * k:16 * (k + 1)] = pr.reshape(4, 16, 7, 7)

    # ---- host: SE MLP ----
    y, v = host_mlp(pooled, w1, w2, wc)
    wpar = float(wp[0])
    b1, maskw, id8, id112 = passB_consts(wpar)

    # ---- pass B: gates + output ----
    nc_b = _get("B", build_passB)
    in_maps_b = []
    for k in cores:
        b, half = k // 2, k % 2
        in_maps_b.append({
            "xb": np.ascontiguousarray(x[b, :, 224 * half:224 * (half + 1), :]),
            "vd8": passB_vd8(v[b]),
            "yrow": y[b][None, :],
            "b1": b1, "maskw": maskw, "id8": id8, "id112": id112,
        })
    res_b = run_bass_kernel_spmd(nc_b, in_maps_b, cores)
    out = np.empty((4, 128, 448, 448), np.float32)
    for k in cores:
        b, half = k // 2, k % 2
        out[b, :, 224 * half:224 * (half + 1), :] = res_b.results[k]["out"]
    return out


# revision 9
# speedup vs baseline: 1.8662x; 1.8662x over previous
"""Trainium2 Bass kernel for nn_CSFI_26182120636676.

FrequencyChannelAttention + FrequencySpatialAttention over x [4, 128, 448, 448] f32.

Two SPMD launches on 8 NeuronCores:
  Pass A (channel-sharded, 16 ch/core): 64x64 block-sum pooling -> [7,7] sums
    per (b, c). DVE segmented reduce over w + tiny PE indicator matmul over h.
    Host finishes the tiny SE MLP (sigmoid gate y[b, c]).
  Pass B (batch x h-half sharded): per 8-row strip, 8 accumulating fp32r
    matmuls fold the channel contraction (with v = y*wc) AND the first 8-pt
    DCT stage; PE transposes move data to w-partitions; a block-diagonal
    matmul applies the second DCT stage; mask*wp multiply; single-column
    transposes collect gate rows at partition 0; ACT sigmoid; rank-1 fp32r
    matmuls broadcast y[c]*gate to 128 partitions; one DVE multiply forms
    out = x * y * gate.
"""
import math
import numpy as np

import concourse.bacc as bacc
import concourse.tile as tile
from concourse import mybir
from concourse.bass_utils import run_bass_kernel_spmd

F32 = mybir.dt.float32
F32R = mybir.dt.float32r

C = 128
H = W = 448
DCT_H = DCT_W = 7
NUM_FREQ = 16
BH = H // DCT_H  # 64

_TOP_X = [0, 0, 6, 0, 0, 1, 1, 4, 5, 1, 3, 0, 0, 0, 3, 2]
_TOP_Y = [0, 1, 0, 5, 2, 0, 2, 0, 0, 6, 0, 4, 6, 3, 5, 2]


def _basis(pos, freq, P):
    v = math.cos(math.pi * freq * (pos + 0.5) / P) / math.sqrt(P)
    return v if freq == 0 else v * math.sqrt(2)


def fca_filter():
    filt = np.zeros((C, DCT_H, DCT_W), np.float32)
    cp = C // NUM_FREQ
    for i, (ux, vy) in enumerate(zip(_TOP_X, _TOP_Y)):
        for xx in range(DCT_H):
            for yy in range(DCT_W):
                filt[i * cp:(i + 1) * cp, xx, yy] = _basis(xx, ux, DCT_H) * _basis(yy, vy, DCT_W)
    return filt


def dct1d(N):
    k = np.arange(N)[:, None].astype(np.float64)
    n = np.arange(N)[None, :].astype(np.float64)
    m = np.cos(np.pi / N * (n + 0.5) * k)
    m[0] /= math.sqrt(N)
    m[1:] /= math.sqrt(N / 2)
    return m.astype(np.float32)


def zigzag_mask(h, w, nf):
    coords = []
    for s in range(h + w - 1):
        if s % 2 == 0:
            r, c = min(s, h - 1), s - min(s, h - 1)
            while r >= 0 and c < w:
                coords.append((r, c)); r -= 1; c += 1
        else:
            c, r = min(s, w - 1), s - min(s, w - 1)
            while c >= 0 and r < h:
                coords.append((r, c)); r += 1; c -= 1
    mask = np.zeros(h * w, np.float32)
    for (r, c) in coords[:nf]:
        mask[r * w + c] = 1.0
    return mask


D8 = dct1d(8)
MASK8 = zigzag_mask(8, 8, NUM_FREQ).reshape(8, 8)
FILT = fca_filter()


def host_mlp(pooled_sums, w1, w2, wc):
    """pooled_sums [4, 128, 7, 7] block SUMS -> gate y [4, 128], v = y*wc [4, 128]."""
    pooled = pooled_sums.astype(np.float32) / (BH * BH)
    y_pre = np.einsum('bchw,chw->bc', pooled, FILT)
    h1 = np.maximum(y_pre @ w1.T, 0.0)
    y = 1.0 / (1.0 + np.exp(-(h1 @ w2.T)))
    v = y * wc[0][None, :]
    return y.astype(np.float32), v.astype(np.float32)


def passA_consts():
    ind = np.zeros((128, 32), np.float32)
    for t in range(4):
        for p in range(128):
            h = 128 * t + p
            if h < H:
                ind[p, t * 8 + (h // BH)] = 1.0
    return ind


def passB_consts(wpar):
    b1 = np.zeros((112, 112), np.float32)
    for wpl in range(14):
        b1[wpl * 8:(wpl + 1) * 8, wpl * 8:(wpl + 1) * 8] = D8.T
    maskw = np.zeros((112, 32), np.float32)
    for wpl in range(14):
        for v in range(8):
            for ch in range(4):
                for u in range(8):
                    maskw[wpl * 8 + v, ch * 8 + u] = wpar * MASK8[u, v]
    id8 = np.eye(8, dtype=np.float32)
    id112 = np.eye(112, dtype=np.float32)
    return b1, maskw, id8, id112


def passB_vd8(v_b):
    out = np.zeros((128, 64), np.float32)
    for ph in range(8):
        out[:, ph * 8:(ph + 1) * 8] = v_b[:, None] * D8[:, ph][None, :]
    return out


def build_passA(n_loop=1):
    nc = bacc.Bacc("TRN2", target_bir_lowering=False, debug=False)
    xa = nc.dram_tensor("xa", [4, 16, H, W], F32, kind="ExternalInput").ap()
    ind_d = nc.dram_tensor("ind", [128, 32], F32, kind="ExternalInput").ap()
    pooled_d = nc.dram_tensor("pooled", [7, 448], F32, kind="ExternalOutput").ap()

    with tile.TileContext(nc) as tc:
        with tc.tile_pool(name="consts", bufs=1) as cpool, \
             tc.tile_pool(name="xin", bufs=3) as xpool, \
             tc.tile_pool(name="xtl", bufs=2) as tpool, \
             tc.tile_pool(name="rs", bufs=2) as rspool, \
             tc.tile_pool(name="psb", bufs=1) as psbpool, \
             tc.tile_pool(name="pp", bufs=2, space="PSUM") as ppool:
            ind = cpool.tile([128, 32], F32)
            nc.sync.dma_start(ind[:], ind_d[:])

            import contextlib
            loop_cm = tc.For_i(0, n_loop, 1) if n_loop > 1 else contextlib.nullcontext()
            with loop_cm:
                psb = psbpool.tile([7, 448], F32, tag="psb")
                # two channel-planes per iteration: planes are DRAM-contiguous
                for it2 in range(32):
                    b, cl = (2 * it2) // 16, (2 * it2) % 16
                    # main rows 0..384 of both planes: [128, (q, t, w)]
                    xt = xpool.tile([128, 2, 3, 448], F32, tag="x")
                    for q in range(2):
                        nc.sync.dma_start(
                            xt[:, q, :, :],
                            xa[b, cl + q, 0:384, :].rearrange(
                                "(t p) w -> p t w", p=128))
                    # tail rows 384..448: [64, (q, w)]
                    xtl = tpool.tile([64, 2, 448], F32, tag="xt")
                    nc.sync.dma_start(
                        xtl[:],
                        xa[b, cl:cl + 2, 384:448, :].rearrange(
                            "q p w -> p q w"))
                    rs = rspool.tile([128, 42], F32, tag="rs")
                    nc.vector.reduce_sum(
                        rs[:],
                        xt[:].rearrange("p q t (j k) -> p (q t) j k", j=7),
                        axis=mybir.AxisListType.X)
                    trs = rspool.tile([64, 14], F32, tag="trs")
                    nc.vector.reduce_sum(
                        trs[:],
                        xtl[:].rearrange("p q (j k) -> p q j k", j=7),
                        axis=mybir.AxisListType.X)
                    pooled_ps = ppool.tile([8, 14], F32, tag="pp")
                    for q in range(2):
                        for t in range(3):
                            nc.tensor.matmul(
                                pooled_ps[:, 7 * q:7 * q + 7],
                                ind[:, 8 * t:8 * t + 8],
                                rs[:, 21 * q + 7 * t:21 * q + 7 * t + 7],
                                start=(t == 0), stop=False)
                        nc.tensor.matmul(
                            pooled_ps[:, 7 * q:7 * q + 7],
                            ind[0:64, 24:32],
                            trs[:, 7 * q:7 * q + 7],
                            start=False, stop=True)
                    nc.scalar.copy(psb[0:7, 14 * it2:14 * it2 + 14],
                                   pooled_ps[0:7, :])
                nc.sync.dma_start(pooled_d[:], psb[:])
    nc.compile()
    return nc


def build_passB(n_strips=28, n_loop=1):
    HS = n_strips * 8
    nc = bacc.Bacc("TRN2", target_bir_lowering=False, debug=False)
    xb = nc.dram_tensor("xb", [128, HS, W], F32R, kind="ExternalInput").ap()
    vd8_d = nc.dram_tensor("vd8", [128, 64], F32R, kind="ExternalInput").ap()
    yrow_d = nc.dram_tensor("yrow", [1, 128], F32R, kind="ExternalInput").ap()
    b1_d = nc.dram_tensor("b1", [112, 112], F32, kind="ExternalInput").ap()
    maskw_d = nc.dram_tensor("maskw", [112, 32], F32, kind="ExternalInput").ap()
    id8_d = nc.dram_tensor("id8", [8, 8], F32, kind="ExternalInput").ap()
    id112_d = nc.dram_tensor("id112", [112, 112], F32, kind="ExternalInput").ap()
    out_d = nc.dram_tensor("out", [128, HS, W], F32, kind="ExternalOutput").ap()

    with tile.TileContext(nc) as tc:
        with tc.tile_pool(name="consts", bufs=1) as cpool, \
             tc.tile_pool(name="xin", bufs=4) as xpool, \
             tc.tile_pool(name="outp", bufs=3) as opool, \
             tc.tile_pool(name="small", bufs=3) as spool, \
             tc.tile_pool(name="gat", bufs=3) as gpool, \
             tc.tile_pool(name="ps_t1", bufs=2, space="PSUM") as ps_t1, \
             tc.tile_pool(name="ps_t1t", bufs=1, space="PSUM") as ps_t1t, \
             tc.tile_pool(name="ps_coef", bufs=1, space="PSUM") as ps_coef, \
             tc.tile_pool(name="ps_frq", bufs=2, space="PSUM") as ps_frq, \
             tc.tile_pool(name="ps_gb", bufs=2, space="PSUM") as ps_gb:
            vd8 = cpool.tile([128, 64], F32R)
            nc.sync.dma_start(vd8[:], vd8_d[:])
            yrow = cpool.tile([1, 128], F32R)
            nc.sync.dma_start(yrow[:], yrow_d[:])
            b1 = cpool.tile([112, 112], F32)
            nc.sync.dma_start(b1[:], b1_d[:])
            maskw = cpool.tile([112, 32], F32)
            nc.sync.dma_start(maskw[:], maskw_d[:])
            id8 = cpool.tile([8, 8], F32)
            nc.sync.dma_start(id8[:], id8_d[:])
            id112 = cpool.tile([112, 112], F32)
            nc.sync.dma_start(id112[:], id112_d[:])

            import contextlib
            loop_cm = tc.For_i(0, n_loop, 1) if n_loop > 1 else contextlib.nullcontext()
            with loop_cm:
                for s in range(n_strips):
                    h0 = 8 * s
                    xt = xpool.tile([128, 8, 448], F32R, tag="x")
                    nc.sync.dma_start(xt[:], xb[:, h0:h0 + 8, :])
                    # T1[u, w] = sum_ph sum_c v[c] D8[u, ph] x[c, ph, w]
                    t1 = ps_t1.tile([8, 448], F32, tag="t1")
                    for ph in range(8):
                        nc.tensor.matmul(
                            t1[:],
                            vd8[:, 8 * ph:8 * ph + 8],
                            xt[:, ph, :],
                            start=(ph == 0), stop=(ph == 7))
                    t1_sb = spool.tile([8, 448], F32, tag="t1sb")
                    nc.scalar.copy(t1_sb[:], t1[:])
                    # transpose chunks -> T1T [112 (wpl,pw), 32 (chunk,u)]
                    t1t = ps_t1t.tile([112, 32], F32, tag="t1t")
                    for ch in range(4):
                        nc.tensor.transpose(
                            t1t[:, 8 * ch:8 * ch + 8],
                            t1_sb[:, 112 * ch:112 * ch + 112], id8[:])
                    t1t_sb = spool.tile([112, 32], F32, tag="t1tsb")
                    nc.vector.tensor_copy(t1t_sb[:], t1t[:])
                    # 2nd DCT stage: COEF [112 (wpl,v), 32 (chunk,u)]
                    coef = ps_coef.tile([112, 32], F32, tag="coef")
                    nc.tensor.matmul(coef[:], b1[:], t1t_sb[:], start=True, stop=True)
                    mc = spool.tile([112, 32], F32, tag="mc")
                    nc.vector.tensor_mul(mc[:], coef[:], maskw[:])
                    # one transpose: FREQT [32 (chunk,u), 112 (wpl,v)]
                    frqt = ps_frq.tile([32, 112], F32, tag="frqt")
                    nc.tensor.transpose(frqt[:], mc[:], id112[:])
                    gate32 = spool.tile([32, 112], F32R, tag="g32")
                    nc.scalar.activation(
                        gate32[:], frqt[:], mybir.ActivationFunctionType.Sigmoid)
                    # SBUF->SBUF DMA: partitions (chunk,u) -> flat gate row
                    # gatesb free order = (chunk, u, wpl*8+v)
                    gatesb = gpool.tile([1, 8 * 448], F32R, tag="gate")
                    nc.scalar.dma_start(
                        gatesb[:].rearrange("p (q f) -> p q f", q=32), gate32[:])
                    gv = gatesb[:].rearrange("p (c u f) -> p c u f", c=4, u=8)
                    # broadcast y[c]*gate to 128 partitions, multiply with x
                    ot = opool.tile([128, 8, 448], F32, tag="o")
                    for u in range(8):
                        gb = ps_gb.tile([128, 448], F32, tag="gb")
                        nc.tensor.matmul(
                            gb[:], yrow[:], gv[:, :, u, :],
                            start=True, stop=True)
                        nc.vector.tensor_mul(
                            ot[:, u, :], xt[:, u, :].bitcast(F32), gb[:])
                    nc.sync.dma_start(out_d[:, h0:h0 + 8, :], ot[:])
    nc.compile()
    return nc


_CACHE = {}


def _get(name, builder):
    if name not in _CACHE:
        _CACHE[name] = builder()
    return _CACHE[name]


def kernel(x, w1, w2, wc, wp):
    x = np.asarray(x, dtype=np.float32)
    w1 = np.asarray(w1, dtype=np.float32)
    w2 = np.asarray(w2, dtype=np.float32)
    wc = np.asarray(wc, dtype=np.float32)
    wp = np.asarray(wp, dtype=np.float32)

    cores = list(range(8))

    # ---- pass A: pooled block sums ----
    nc_a = _get("A", build_passA)
    ind = passA_consts()
    in_maps_a = [
        {"xa": np.ascontiguousarray(x[:, 16 * k:16 * (k + 1)]), "ind": ind}
        for k in cores
    ]
    res_a = run_bass_kernel_spmd(nc_a, in_maps_a, cores)
    pooled = np.zeros((4, 128, 7, 7), np.float32)
    for k in cores:
        pr = res_a.results[k]["pooled"].reshape(7, 64, 7).transpose(1, 0, 2)  # [pair, i, j]
        pooled[:, 16 * k:16 * (k + 1)] = pr.reshape(4, 16, 7, 7)

    # ---- host: SE MLP ----
    y, v = host_mlp(pooled, w1, w2, wc)
    wpar = float(wp[0])
    b1, maskw, id8, id112 = passB_consts(wpar)

    # ---- pass B: gates + output ----
    nc_b = _get("B", build_passB)
    in_maps_b = []
    for k in cores:
        b, half = k // 2, k % 2
        in_maps_b.append({
            "xb": np.ascontiguousarray(x[b, :, 224 * half:224 * (half + 1), :]),
            "vd8": passB_vd8(v[b]),
            "yrow": y[b][None, :],
            "b1": b1, "maskw": maskw, "id8": id8, "id112": id112,
        })
    res_b = run_bass_kernel_spmd(nc_b, in_maps_b, cores)
    out = np.empty((4, 128, 448, 448), np.float32)
    for k in cores:
        b, half = k // 2, k % 2
        out[b, :, 224 * half:224 * (half + 1), :] = res_b.results[k]["out"]
    return out
